# revision 1
# baseline (speedup 1.0000x reference)
"""CrossViewAttention Trainium2 kernel.

Shards the B*V=16 (batch, view) attention instances across 8 NeuronCores
(2 per core, data-parallel; weights replicated). The circular neighbor
gather (views v-1, v+1) is resolved on the host when slicing per-core
inputs, so no device collectives are needed.

Per core, for each of its 2 pairs:
  Q^T = wq.T @ x^T           (fp32r matmuls, d-contraction)
  K^T = wk.T @ x_kv^T        V = x_kv @ wv  (natural layout, +ones col)
  scores^T[t,s] = K^T.T @ Q^T   per head (GQA: head h uses kv head h//4)
  E = exp(scale*scores^T)    (no max subtraction; scores are O(1))
  [O^T; l] = V_aug.T @ E     (ones column folds the softmax denominator)
  O^T *= broadcast(1/l)      (K=1 ones matmul broadcasts 1/l over hd)
  y = O @ wo
"""
import numpy as np

B, V, S, D = 2, 8, 256, 2048
NH, NKV, KVR = 32, 8, 2
HD = D // NH  # 64
G = NH // NKV  # 4
N_CORES = 8
P = 2  # pairs per core
SCALE = 1.0 / np.sqrt(HD)

_CACHE = {}


def _to_f32r(a: np.ndarray) -> np.ndarray:
    """Round fp32 to the fp32r format (e8m11, RNE): low 12 bits zeroed."""
    u = np.ascontiguousarray(a, dtype=np.float32).view(np.uint32)
    u = (u + 0x7FF + ((u >> 12) & 1)) & 0xFFFFF000
    return u.view(np.float32)


def _build():
    import concourse.bass as bass
    import concourse.tile as tile
    import concourse.mybir as mybir
    from concourse import bacc
    from contextlib import ExitStack

    F32 = mybir.dt.float32
    F32R = mybir.dt.float32r

    nc = bacc.Bacc("TRN2", target_bir_lowering=False, debug=False,
                   num_devices=N_CORES)
    xqT = nc.dram_tensor("xqT", [D, P * S], F32R, kind="ExternalInput").ap()
    xkvT = nc.dram_tensor("xkvT", [D, P * 512], F32R, kind="ExternalInput").ap()
    wq = nc.dram_tensor("wq", [D, D], F32R, kind="ExternalInput").ap()
    wkv = nc.dram_tensor("wkv", [D, 1024], F32R, kind="ExternalInput").ap()
    wo = nc.dram_tensor("wo", [D, D], F32R, kind="ExternalInput").ap()
    ones1 = nc.dram_tensor("ones1", [1, HD], F32R, kind="ExternalInput").ap()
    vones = nc.dram_tensor("vones", [128, 8], F32R, kind="ExternalInput").ap()
    y = nc.dram_tensor("y", [P * S, D], F32, kind="ExternalOutput").ap()

    with tile.TileContext(nc) as tc, ExitStack() as top:
        misc = top.enter_context(tc.tile_pool(name="misc", bufs=2))
        ktp = top.enter_context(tc.tile_pool(name="ktp", bufs=1))
        vp = top.enter_context(tc.tile_pool(name="vp", bufs=1))

        on_sb = misc.tile([1, HD], F32R, tag="ones")
        nc.gpsimd.dma_start(on_sb[:], ones1[:])
        vo_sb = misc.tile([128, 8], F32R, tag="vones")
        nc.gpsimd.dma_start(vo_sb[:], vones[:])

        KT = [ktp.tile([64, 2048], F32R, tag=f"kt{i}", name=f"kt{i}") for i in range(4)]
        VA = [[vp.tile([128, 8 * 65], F32R, tag=f"va{p}_{t}", name=f"va{p}_{t}") for t in range(4)]
              for p in range(P)]

        # ---------- Phase A1/A2: K^T, V (uses xkvT; wv resident) ----------
        with ExitStack() as ph:
            xkp = ph.enter_context(tc.tile_pool(name="xkp", bufs=1))
            wvp = ph.enter_context(tc.tile_pool(name="wvp", bufs=6))
            wst = ph.enter_context(tc.tile_pool(name="wst", bufs=6))
            psA = ph.enter_context(tc.tile_pool(name="psA", bufs=8, space="PSUM"))

            xkv = []
            for k in range(16):
                t = xkp.tile([128, 1024], F32R, tag=f"xkv{k}", name=f"xkv{k}")
                nc.sync.dma_start(t[:], xkvT[k * 128:(k + 1) * 128, :])
                xkv.append(t)

            # A1: K^T[f, t]; k outer, batched wk loads, 8 accumulators
            kps = [psA.tile([128, 512], F32, tag="pa", name=f"kps{i}")
                   for i in range(8)]
            for k in range(16):
                wt = wst.tile([128, 512], F32R, tag="wk")
                nc.sync.dma_start(wt[:], wkv[k * 128:(k + 1) * 128, 0:512])
                for fk in range(4):
                    nc.tensor.matmul(kps[fk * 2][:],
                                     wt[:, fk * 128:(fk + 1) * 128],
                                     xkv[k][:, 0:512],
                                     start=(k == 0), stop=(k == 15))
                    nc.tensor.matmul(kps[fk * 2 + 1][:],
                                     wt[:, fk * 128:(fk + 1) * 128],
                                     xkv[k][:, 512:1024],
                                     start=(k == 0), stop=(k == 15))
            for fk in range(4):
                ps0, ps1 = kps[fk * 2], kps[fk * 2 + 1]
                nc.vector.tensor_copy(KT[fk][0:64, 0:512], ps0[0:64, :])
                nc.vector.tensor_copy(KT[fk][0:64, 1024:1536], ps0[64:128, :])
                nc.vector.tensor_copy(KT[fk][0:64, 512:1024], ps1[0:64, :])
                nc.vector.tensor_copy(KT[fk][0:64, 1536:2048], ps1[64:128, :])

            # A2: V natural [t, f] + ones; k-outer per pair, wv streamed 2x
            for p in range(P):
                vps = [psA.tile([128, 512], F32, tag="pa", name=f"pvv{p}_{i}")
                       for i in range(4)]
                for k in range(16):
                    wvt = wvp.tile([128, 512], F32R, tag="wv")
                    nc.sync.dma_start(wvt[:], wkv[k * 128:(k + 1) * 128, 512:1024])
                    for tt in range(4):
                        nc.tensor.matmul(
                            vps[tt][:],
                            xkv[k][:, p * 512 + tt * 128:p * 512 + (tt + 1) * 128],
                            wvt[:],
                            start=(k == 0), stop=(k == 15))
                for tt in range(4):
                    ps = vps[tt]
                    va = VA[p][tt]
                    dst = va[:].rearrange("q (h c) -> q h c", c=65)[:, :, 0:64]
                    src = ps[:].rearrange("q (h c) -> q h c", c=64)
                    nc.vector.tensor_copy(dst, src)
                    od = va[:].rearrange("q (h c) -> q h c", c=65)[:, :, 64:65]
                    nc.vector.tensor_copy(od, vo_sb[:].unsqueeze(2))

        # ---------- Phase A3: Q^T (uses xqT) ----------
        qtp = top.enter_context(tc.tile_pool(name="qtp", bufs=1))
        QT = [qtp.tile([64, 1024], F32R, tag=f"qt{j}", name=f"qt{j}")
              for j in range(16)]
        otp = top.enter_context(tc.tile_pool(name="otp", bufs=1))
        OT = [[otp.tile([128, 256], F32R, tag=f"ot{p}_{i}", name=f"ot{p}_{i}") for i in range(16)]
              for p in range(P)]

        with ExitStack() as ph:
            xqp = ph.enter_context(tc.tile_pool(name="xqp", bufs=1))
            wst = ph.enter_context(tc.tile_pool(name="wst2", bufs=6))
            psA = ph.enter_context(tc.tile_pool(name="psA2", bufs=8, space="PSUM"))

            xq = []
            for k in range(16):
                t = xqp.tile([128, 512], F32R, tag=f"xq{k}", name=f"xq{k}")
                nc.sync.dma_start(t[:], xqT[k * 128:(k + 1) * 128, :])
                xq.append(t)

            for fg in range(4):
                qps = [psA.tile([128, 512], F32, tag="pa", name=f"qps{fg}_{i}")
                       for i in range(4)]
                for k in range(16):
                    wt = wst.tile([128, 512], F32R, tag="wq")
                    nc.sync.dma_start(
                        wt[:], wq[k * 128:(k + 1) * 128, fg * 512:(fg + 1) * 512])
                    for fi in range(4):
                        nc.tensor.matmul(qps[fi][:],
                                         wt[:, fi * 128:(fi + 1) * 128],
                                         xq[k][:],
                                         start=(k == 0), stop=(k == 15))
                for fi in range(4):
                    fq = fg * 4 + fi
                    ps = qps[fi]
                    nc.vector.tensor_copy(QT[fq][0:64, 0:512], ps[0:64, :])
                    nc.vector.tensor_copy(QT[fq][0:64, 512:1024], ps[64:128, :])

        # ---------- Phase B: attention ----------
        with ExitStack() as ph:
            ep = ph.enter_context(tc.tile_pool(name="ep", bufs=8))
            lp = ph.enter_context(tc.tile_pool(name="lp", bufs=4))
            rp = ph.enter_context(tc.tile_pool(name="rp", bufs=4))
            qkps = ph.enter_context(tc.tile_pool(name="qkps", bufs=5, space="PSUM"))
            pvps = ph.enter_context(tc.tile_pool(name="pvps", bufs=2, space="PSUM"))
            rps = ph.enter_context(tc.tile_pool(name="rps", bufs=1, space="PSUM"))

            for j in range(16):
                n = j // 2
                u = j % 2
                jk = n // 2
                nhalf = n % 2
                hA = 4 * n + 2 * u
                hB = hA + 1
                for p in range(P):
                    # QK^T: per t-tile, heads A (rows 0:64) and B (64:128)
                    es = []
                    for tt in range(4):
                        qk = qkps.tile([128, 512], F32, tag="qk")
                        cA = nhalf * 1024 + p * 512 + tt * 128
                        nc.tensor.matmul(
                            qk[:, 0:256],
                            KT[jk][0:64, cA:cA + 128],
                            QT[j][0:64, p * 256:(p + 1) * 256],
                            start=True, stop=True)
                        nc.tensor.matmul(
                            qk[:, 256:512],
                            KT[jk][0:64, cA:cA + 128],
                            QT[j][0:64, 512 + p * 256:512 + (p + 1) * 256],
                            start=True, stop=True)
                        e = ep.tile([128, 512], F32R, tag="e")
                        nc.scalar.activation(
                            e[:], qk[:], mybir.ActivationFunctionType.Exp,
                            scale=float(SCALE))
                        es.append(e)

                    # PV with ones-fold: [65, 512] = [O^T_A | O^T_B ; l]
                    pv = pvps.tile([65, 512], F32, tag="pv")
                    for tt in range(4):
                        nc.tensor.matmul(
                            pv[:, 0:512],
                            VA[p][tt][:, n * 65:(n + 1) * 65],
                            es[tt][:, 0:512],
                            start=(tt == 0), stop=(tt == 3))

                    # softmax denominators -> broadcast reciprocal
                    l2 = lp.tile([1, 512], F32, tag="l2")
                    nc.vector.tensor_copy(l2[0:1, 0:256], pv[64:65, 0:256])
                    nc.vector.tensor_copy(l2[0:1, 256:512], pv[64:65, 256:512])
                    r2 = lp.tile([1, 512], F32R, tag="r2")
                    with nc.allow_low_precision(reason="fp32r matmul input"):
                        nc.vector.reciprocal(r2[:], l2[:])
                    rb = rps.tile([64, 512], F32, tag="rb")
                    nc.tensor.matmul(rb[:, 0:256], on_sb[:], r2[0:1, 0:256],
                                     start=True, stop=True)
                    nc.tensor.matmul(rb[:, 256:512], on_sb[:], r2[0:1, 256:512],
                                     start=True, stop=True)
                    rsb = rp.tile([64, 512], F32, tag="rsb")
                    nc.vector.tensor_copy(rsb[:], rb[:])

                    # normalize + scatter to O^T tiles
                    for h, c0 in ((hA, 0), (hB, 256)):
                        ot = OT[p][h // 2]
                        ob = (h % 2) * 64
                        nc.vector.tensor_tensor(
                            ot[ob:ob + 64, :],
                            pv[0:64, c0:c0 + 256],
                            rsb[0:64, c0:c0 + 256],
                            mybir.AluOpType.mult)

        # ---------- Phase C: output projection ----------
        with ExitStack() as ph:
            wop = ph.enter_context(tc.tile_pool(name="wop", bufs=6))
            yst = ph.enter_context(tc.tile_pool(name="yst", bufs=4))
            psC = ph.enter_context(tc.tile_pool(name="psC", bufs=8, space="PSUM"))

            for nn in range(4):
                acc = [[psC.tile([128, 512], F32, tag="pc", name=f"pc{nn}_{m}") for m in range(2)]
                       for p in range(P)]
                for k in range(16):
                    wt = wop.tile([128, 512], F32R, tag="wo")
                    nc.sync.dma_start(
                        wt[:], wo[k * 128:(k + 1) * 128, nn * 512:(nn + 1) * 512])
                    for p in range(P):
                        for m in range(2):
                            nc.tensor.matmul(
                                acc[p][m][:],
                                OT[p][k][:, m * 128:(m + 1) * 128],
                                wt[:],
                                start=(k == 0), stop=(k == 15))
                for p in range(P):
                    for m in range(2):
                        yt = yst.tile([128, 512], F32, tag="yt")
                        nc.vector.tensor_copy(yt[:], acc[p][m][:])
                        r0 = p * 256 + m * 128
                        nc.sync.dma_start(
                            y[r0:r0 + 128, nn * 512:(nn + 1) * 512], yt[:])

    nc.compile()
    return nc


def _get_nc():
    if "nc" not in _CACHE:
        _CACHE["nc"] = _build()
    return _CACHE["nc"]


def kernel(x, wq, wkv, wo):
    from concourse.bass_utils import run_bass_kernel_spmd

    nc = _get_nc()
    x = np.asarray(x, dtype=np.float32)
    wq_r = _to_f32r(wq)
    wkv_r = _to_f32r(wkv)
    wo_r = _to_f32r(wo)
    ones1 = np.ones((1, HD), np.float32)
    vones = np.ones((128, 8), np.float32)

    in_maps = []
    for c in range(N_CORES):
        xq_cols = []
        xkv_cols = []
        for p in range(P):
            pg = 2 * c + p
            b, v = pg // V, pg % V
            xq_cols.append(np.ascontiguousarray(x[b, v].T))
            xkv_cols.append(np.ascontiguousarray(
                np.concatenate([x[b, (v - 1) % V], x[b, (v + 1) % V]], axis=0).T))
        in_maps.append({
            "xqT": _to_f32r(np.concatenate(xq_cols, axis=1)),
            "xkvT": _to_f32r(np.concatenate(xkv_cols, axis=1)),
            "wq": wq_r, "wkv": wkv_r, "wo": wo_r, "ones1": ones1, "vones": vones,
        })

    res = run_bass_kernel_spmd(nc, in_maps, list(range(N_CORES)),
                               trace=False)
    out = np.empty((B, V, S, D), np.float32)
    for c in range(N_CORES):
        yc = res.results[c]["y"]
        for p in range(P):
            pg = 2 * c + p
            b, v = pg // V, pg % V
            out[b, v] = yc[p * S:(p + 1) * S]
    return out



# revision 4
# speedup vs baseline: 1.0510x; 1.0510x over previous
"""CrossViewAttention Trainium2 kernel (bf16).

Shards the B*V=16 (batch, view) attention instances across 8 NeuronCores
(2 per core, data-parallel; weights replicated). The circular neighbor
gather (views v-1, v+1) is resolved on the host when slicing per-core
inputs, so no device collectives are needed.

All matmul operands are bf16 (PSUM accumulation stays fp32), which
halves HBM traffic and weight-load time vs fp32r at the same 1
cycle/row multiply rate. Per core, for each of its 2 pairs:
  K^T = wk.T @ x_kv^T        V = x_kv @ wv  (natural layout, +ones col)
  Q^T = wq.T @ x^T
  scores^T[t,q] = K^T.T @ Q^T   per head (GQA: head h uses kv head h//4)
  E = exp(scale*scores^T)    (no max subtraction; scores are O(1))
  [O^T; l] = V_aug.T @ E     (ones column folds the softmax denominator)
  O^T *= broadcast(1/l)      (K=1 ones matmul broadcasts 1/l over hd)
  y = O @ wo
"""
import numpy as np

B, V, S, D = 2, 8, 256, 2048
NH, NKV, KVR = 32, 8, 2
HD = D // NH  # 64
G = NH // NKV  # 4
N_CORES = 8
P = 2  # pairs per core
SCALE = 1.0 / np.sqrt(HD)

_CACHE = {}


def _build():
    import concourse.bass as bass
    import concourse.tile as tile
    import concourse.mybir as mybir
    from concourse import bacc
    from contextlib import ExitStack

    F32 = mybir.dt.float32
    F32R = mybir.dt.float32r
    BF16 = mybir.dt.bfloat16

    nc = bacc.Bacc("TRN2", target_bir_lowering=False, debug=False,
                   num_devices=N_CORES)
    xqT = nc.dram_tensor("xqT", [D, P * S], BF16, kind="ExternalInput").ap()
    xkvT = nc.dram_tensor("xkvT", [D, P * 512], BF16, kind="ExternalInput").ap()
    wq = nc.dram_tensor("wq", [D, D], BF16, kind="ExternalInput").ap()
    wkv = nc.dram_tensor("wkv", [D, 1024], BF16, kind="ExternalInput").ap()
    wo = nc.dram_tensor("wo", [D, D], BF16, kind="ExternalInput").ap()
    ones1 = nc.dram_tensor("ones1", [1, HD], F32R, kind="ExternalInput").ap()
    y = nc.dram_tensor("y", [P * S, D], F32, kind="ExternalOutput").ap()

    with tile.TileContext(nc) as tc, ExitStack() as top:
        misc = top.enter_context(tc.tile_pool(name="misc", bufs=1))
        ktp = top.enter_context(tc.tile_pool(name="ktp", bufs=1))
        vp = top.enter_context(tc.tile_pool(name="vp", bufs=1))

        # ones row for the 1/l broadcast matmul (f32r to pair with f32r recip)
        on_sb = misc.tile([1, HD], F32R, tag="ones")
        nc.gpsimd.dma_start(on_sb[:], ones1[:])

        KT = [ktp.tile([64, 2048], BF16, tag=f"kt{i}", name=f"kt{i}") for i in range(4)]
        VA = [[vp.tile([128, 8 * 65], BF16, tag=f"va{p}_{t}", name=f"va{p}_{t}") for t in range(4)]
              for p in range(P)]
        # softmax-denominator ones columns of V_aug: set once
        for p in range(P):
            for t in range(4):
                od = VA[p][t][:].rearrange("q (h c) -> q h c", c=65)[:, :, 64:65]
                nc.gpsimd.memset(od, 1.0)

        # ---------- Phase A1/A2: K^T, V (uses xkvT; xkv resident) ----------
        with ExitStack() as ph:
            xkp = ph.enter_context(tc.tile_pool(name="xkp", bufs=1))
            wvp = ph.enter_context(tc.tile_pool(name="wvp", bufs=6))
            wst = ph.enter_context(tc.tile_pool(name="wst", bufs=6))
            psA = ph.enter_context(tc.tile_pool(name="psA", bufs=8, space="PSUM"))

            xkv = []
            for k in range(16):
                t = xkp.tile([128, 1024], BF16, tag=f"xkv{k}", name=f"xkv{k}")
                nc.sync.dma_start(t[:], xkvT[k * 128:(k + 1) * 128, :])
                xkv.append(t)

            # A1: K^T[f, t]; k outer, batched wk loads, 8 accumulators
            kps = [psA.tile([128, 512], F32, tag="pa", name=f"kps{i}")
                   for i in range(8)]
            for k in range(16):
                wt = wst.tile([128, 512], BF16, tag="wk")
                nc.sync.dma_start(wt[:], wkv[k * 128:(k + 1) * 128, 0:512])
                for fk in range(4):
                    nc.tensor.matmul(kps[fk * 2][:],
                                     wt[:, fk * 128:(fk + 1) * 128],
                                     xkv[k][:, 0:512],
                                     start=(k == 0), stop=(k == 15))
                    nc.tensor.matmul(kps[fk * 2 + 1][:],
                                     wt[:, fk * 128:(fk + 1) * 128],
                                     xkv[k][:, 512:1024],
                                     start=(k == 0), stop=(k == 15))
            for fk in range(4):
                ps0, ps1 = kps[fk * 2], kps[fk * 2 + 1]
                nc.vector.tensor_copy(KT[fk][0:64, 0:512], ps0[0:64, :])
                nc.scalar.copy(KT[fk][0:64, 1024:1536], ps0[64:128, :])
                nc.vector.tensor_copy(KT[fk][0:64, 512:1024], ps1[0:64, :])
                nc.scalar.copy(KT[fk][0:64, 1536:2048], ps1[64:128, :])

            # A2: V natural [t, f]; shared wv load per k, 8 accumulators
            vps = [[psA.tile([128, 512], F32, tag="pa", name=f"pvv{p}_{i}")
                    for i in range(4)] for p in range(P)]
            for k in range(16):
                wvt = wvp.tile([128, 512], BF16, tag="wv")
                nc.sync.dma_start(wvt[:], wkv[k * 128:(k + 1) * 128, 512:1024])
                for p in range(P):
                    for tt in range(4):
                        nc.tensor.matmul(
                            vps[p][tt][:],
                            xkv[k][:, p * 512 + tt * 128:p * 512 + (tt + 1) * 128],
                            wvt[:],
                            start=(k == 0), stop=(k == 15))
            for p in range(P):
                for tt in range(4):
                    ps = vps[p][tt]
                    va = VA[p][tt]
                    dst = va[:].rearrange("q (h c) -> q h c", c=65)[:, :, 0:64]
                    src = ps[:].rearrange("q (h c) -> q h c", c=64)
                    if (p * 4 + tt) % 2 == 0:
                        nc.vector.tensor_copy(dst, src)
                    else:
                        nc.scalar.copy(dst, src)

        # ---------- Phase A3: Q^T (uses xqT) ----------
        qtp = top.enter_context(tc.tile_pool(name="qtp", bufs=1))
        QT = [qtp.tile([64, 1024], BF16, tag=f"qt{j}", name=f"qt{j}")
              for j in range(16)]
        otp = top.enter_context(tc.tile_pool(name="otp", bufs=1))
        OT = [[otp.tile([128, 256], BF16, tag=f"ot{p}_{i}", name=f"ot{p}_{i}") for i in range(16)]
              for p in range(P)]

        with ExitStack() as ph:
            xqp = ph.enter_context(tc.tile_pool(name="xqp", bufs=1))
            wst = ph.enter_context(tc.tile_pool(name="wst2", bufs=6))
            psA = ph.enter_context(tc.tile_pool(name="psA2", bufs=8, space="PSUM"))

            xq = []
            for k in range(16):
                t = xqp.tile([128, 512], BF16, tag=f"xq{k}", name=f"xq{k}")
                nc.sync.dma_start(t[:], xqT[k * 128:(k + 1) * 128, :])
                xq.append(t)

            for fg in range(4):
                qps = [psA.tile([128, 512], F32, tag="pa", name=f"qps{fg}_{i}")
                       for i in range(4)]
                for k in range(16):
                    wt = wst.tile([128, 512], BF16, tag="wq")
                    nc.sync.dma_start(
                        wt[:], wq[k * 128:(k + 1) * 128, fg * 512:(fg + 1) * 512])
                    for fi in range(4):
                        nc.tensor.matmul(qps[fi][:],
                                         wt[:, fi * 128:(fi + 1) * 128],
                                         xq[k][:],
                                         start=(k == 0), stop=(k == 15))
                for fi in range(4):
                    fq = fg * 4 + fi
                    ps = qps[fi]
                    nc.vector.tensor_copy(QT[fq][0:64, 0:512], ps[0:64, :])
                    nc.scalar.copy(QT[fq][0:64, 512:1024], ps[64:128, :])

        # ---------- Phase B: attention ----------
        with ExitStack() as ph:
            ep = ph.enter_context(tc.tile_pool(name="ep", bufs=8))
            lp = ph.enter_context(tc.tile_pool(name="lp", bufs=4))
            qkps = ph.enter_context(tc.tile_pool(name="qkps", bufs=4, space="PSUM"))
            pvps = ph.enter_context(tc.tile_pool(name="pvps", bufs=2, space="PSUM"))
            rps = ph.enter_context(tc.tile_pool(name="rps", bufs=2, space="PSUM"))

            # deferred normalize state from the previous (j, p) iteration
            pending = []

            def finish(st):
                pv, r2, j, p = st
                rb = rps.tile([64, 512], F32, tag="rb")
                nc.tensor.matmul(rb[:, 0:256], on_sb[:], r2[0:1, 0:256],
                                 start=True, stop=True)
                nc.tensor.matmul(rb[:, 256:512], on_sb[:], r2[0:1, 256:512],
                                 start=True, stop=True)
                rsb = lp.tile([64, 512], F32, tag="rsb")
                nc.scalar.copy(rsb[:], rb[:])
                hA, hB = 2 * j, 2 * j + 1
                for h, c0 in ((hA, 0), (hB, 256)):
                    ot = OT[p][h // 2]
                    ob = (h % 2) * 64
                    nc.vector.tensor_tensor(
                        ot[ob:ob + 64, :],
                        pv[0:64, c0:c0 + 256],
                        rsb[0:64, c0:c0 + 256],
                        mybir.AluOpType.mult)

            for j in range(16):
                n = j // 2          # kv head
                jk = n // 2         # KT tile
                nhalf = n % 2       # kv-head half within KT tile
                for p in range(P):
                    pv = pvps.tile([65, 512], F32, tag="pv")
                    es = []
                    for tt in range(4):
                        qk = qkps.tile([128, 512], F32, tag="qk")
                        cA = nhalf * 1024 + p * 512 + tt * 128
                        # both grouped heads (2j, 2j+1) in one matmul
                        rhs = QT[j][:].rearrange(
                            "h (two q) -> h two q", two=2)[:, :, p * 256:(p + 1) * 256]
                        nc.tensor.matmul(
                            qk[:],
                            KT[jk][0:64, cA:cA + 128],
                            rhs,
                            start=True, stop=True)
                        e = ep.tile([128, 512], BF16, tag="e")
                        nc.scalar.activation(
                            e[:], qk[:], mybir.ActivationFunctionType.Exp,
                            scale=float(SCALE))
                        es.append(e)
                        if tt >= 1:
                            # PV lags QK by one t-tile so exp can overlap
                            nc.tensor.matmul(
                                pv[:, 0:512],
                                VA[p][tt - 1][:, n * 65:(n + 1) * 65],
                                es[tt - 1][:, 0:512],
                                start=(tt == 1), stop=False)
                    if pending:
                        finish(pending.pop())
                    nc.tensor.matmul(
                        pv[:, 0:512],
                        VA[p][3][:, n * 65:(n + 1) * 65],
                        es[3][:, 0:512],
                        start=False, stop=True)

                    # softmax denominators -> reciprocal (row layout)
                    l2 = lp.tile([1, 512], F32, tag="l2")
                    nc.vector.tensor_copy(l2[0:1, 0:256], pv[64:65, 0:256])
                    nc.vector.tensor_copy(l2[0:1, 256:512], pv[64:65, 256:512])
                    r2 = lp.tile([1, 512], F32R, tag="r2")
                    with nc.allow_low_precision(reason="fp32r matmul input"):
                        nc.vector.reciprocal(r2[:], l2[:])
                    pending.append((pv, r2, j, p))
            finish(pending.pop())

        # ---------- Phase C: output projection ----------
        with ExitStack() as ph:
            wop = ph.enter_context(tc.tile_pool(name="wop", bufs=6))
            yst = ph.enter_context(tc.tile_pool(name="yst", bufs=4))
            psC = ph.enter_context(tc.tile_pool(name="psC", bufs=8, space="PSUM"))

            for nn in range(4):
                acc = [[psC.tile([128, 512], F32, tag="pc", name=f"pc{nn}_{m}") for m in range(2)]
                       for p in range(P)]
                for k in range(16):
                    wt = wop.tile([128, 512], BF16, tag="wo")
                    nc.sync.dma_start(
                        wt[:], wo[k * 128:(k + 1) * 128, nn * 512:(nn + 1) * 512])
                    for p in range(P):
                        for m in range(2):
                            nc.tensor.matmul(
                                acc[p][m][:],
                                OT[p][k][:, m * 128:(m + 1) * 128],
                                wt[:],
                                start=(k == 0), stop=(k == 15))
                for p in range(P):
                    for m in range(2):
                        yt = yst.tile([128, 512], F32, tag="yt")
                        if m == 0:
                            nc.vector.tensor_copy(yt[:], acc[p][m][:])
                        else:
                            nc.scalar.copy(yt[:], acc[p][m][:])
                        r0 = p * 256 + m * 128
                        nc.sync.dma_start(
                            y[r0:r0 + 128, nn * 512:(nn + 1) * 512], yt[:])

    nc.compile()
    return nc


def _get_nc():
    if "nc" not in _CACHE:
        _CACHE["nc"] = _build()
    return _CACHE["nc"]


def _shard_inputs(x, wq, wkv, wo):
    import ml_dtypes

    bf16 = ml_dtypes.bfloat16
    x = np.asarray(x, dtype=np.float32)
    wq_b = np.ascontiguousarray(wq, dtype=np.float32).astype(bf16)
    wkv_b = np.ascontiguousarray(wkv, dtype=np.float32).astype(bf16)
    wo_b = np.ascontiguousarray(wo, dtype=np.float32).astype(bf16)

    in_maps = []
    for c in range(N_CORES):
        xq_cols = []
        xkv_cols = []
        for p in range(P):
            pg = 2 * c + p
            b, v = pg // V, pg % V
            xq_cols.append(np.ascontiguousarray(x[b, v].T))
            xkv_cols.append(np.ascontiguousarray(
                np.concatenate([x[b, (v - 1) % V], x[b, (v + 1) % V]], axis=0).T))
        in_maps.append({
            "xqT": np.concatenate(xq_cols, axis=1).astype(bf16),
            "xkvT": np.concatenate(xkv_cols, axis=1).astype(bf16),
            "wq": wq_b, "wkv": wkv_b, "wo": wo_b,
            "ones1": np.ones((1, HD), np.float32),
        })
    return in_maps


def kernel(x, wq, wkv, wo):
    from concourse.bass_utils import run_bass_kernel_spmd

    nc = _get_nc()
    in_maps = _shard_inputs(x, wq, wkv, wo)
    res = run_bass_kernel_spmd(nc, in_maps, list(range(N_CORES)),
                               trace=False)
    out = np.empty((B, V, S, D), np.float32)
    for c in range(N_CORES):
        yc = res.results[c]["y"]
        for p in range(P):
            pg = 2 * c + p
            b, v = pg // V, pg % V
            out[b, v] = yc[p * S:(p + 1) * S]
    return out


# revision 6
# speedup vs baseline: 1.2583x; 1.1973x over previous
"""CrossViewAttention Trainium2 kernel (bf16).

Shards the B*V=16 (batch, view) attention instances across 8 NeuronCores
(2 per core, data-parallel; weights replicated). The circular neighbor
gather (views v-1, v+1) is resolved on the host when slicing per-core
inputs, so no device collectives are needed.

All matmul operands are bf16 (PSUM accumulation stays fp32), which
halves HBM traffic and weight-load time vs fp32r at the same 1
cycle/row multiply rate. Per core, for each of its 2 pairs:
  K^T = wk.T @ x_kv^T        V = x_kv @ wv  (natural layout, +ones col)
  Q^T = wq.T @ x^T
  scores^T[t,q] = K^T.T @ Q^T   per head (GQA: head h uses kv head h//4)
  E = exp(scale*scores^T)    (no max subtraction; scores are O(1))
  [O^T; l] = V_aug.T @ E     (ones column folds the softmax denominator)
  O^T *= broadcast(1/l)      (K=1 ones matmul broadcasts 1/l over hd)
  y = O @ wo
"""
import numpy as np

B, V, S, D = 2, 8, 256, 2048
NH, NKV, KVR = 32, 8, 2
HD = D // NH  # 64
G = NH // NKV  # 4
N_CORES = 8
P = 2  # pairs per core
SCALE = 1.0 / np.sqrt(HD)

_CACHE = {}


def _build():
    import concourse.bass as bass
    import concourse.tile as tile
    import concourse.mybir as mybir
    from concourse import bacc
    from contextlib import ExitStack

    F32 = mybir.dt.float32
    F32R = mybir.dt.float32r
    BF16 = mybir.dt.bfloat16

    nc = bacc.Bacc("TRN2", target_bir_lowering=False, debug=False,
                   num_devices=N_CORES)
    xqT = nc.dram_tensor("xqT", [D, P * S], BF16, kind="ExternalInput").ap()
    xkvT = nc.dram_tensor("xkvT", [D, P * 512], BF16, kind="ExternalInput").ap()
    wq = nc.dram_tensor("wq", [D, D], BF16, kind="ExternalInput").ap()
    wkv = nc.dram_tensor("wkv", [D, 1024], BF16, kind="ExternalInput").ap()
    wo = nc.dram_tensor("wo", [D, D], BF16, kind="ExternalInput").ap()
    ones1 = nc.dram_tensor("ones1", [1, HD], F32R, kind="ExternalInput").ap()
    y = nc.dram_tensor("y", [P * S, D], F32, kind="ExternalOutput").ap()

    with tile.TileContext(nc) as tc, ExitStack() as top:
        misc = top.enter_context(tc.tile_pool(name="misc", bufs=1))
        ktp = top.enter_context(tc.tile_pool(name="ktp", bufs=1))
        vp = top.enter_context(tc.tile_pool(name="vp", bufs=1))

        # ones row for the 1/l broadcast matmul (f32r to pair with f32r recip)
        on_sb = misc.tile([1, HD], F32R, tag="ones")
        nc.gpsimd.dma_start(on_sb[:], ones1[:])

        KT = [ktp.tile([64, 2048], BF16, tag=f"kt{i}", name=f"kt{i}") for i in range(4)]
        VA = [[vp.tile([128, 8 * 65], BF16, tag=f"va{p}_{t}", name=f"va{p}_{t}") for t in range(4)]
              for p in range(P)]
        # softmax-denominator ones columns of V_aug: set once
        for p in range(P):
            for t in range(4):
                od = VA[p][t][:].rearrange("q (h c) -> q h c", c=65)[:, :, 64:65]
                nc.gpsimd.memset(od, 1.0)

        # ---------- Phase A1/A2: K^T, V (uses xkvT; xkv resident) ----------
        with ExitStack() as ph:
            xkp = ph.enter_context(tc.tile_pool(name="xkp", bufs=1))
            wvp = ph.enter_context(tc.tile_pool(name="wvp", bufs=6))
            wst = ph.enter_context(tc.tile_pool(name="wst", bufs=6))
            psA = ph.enter_context(tc.tile_pool(name="psA", bufs=8, space="PSUM"))

            xkv = []
            for k in range(16):
                t = xkp.tile([128, 1024], BF16, tag=f"xkv{k}", name=f"xkv{k}")
                nc.sync.dma_start(t[:], xkvT[k * 128:(k + 1) * 128, :])
                xkv.append(t)

            # A1: K^T[f, t]; k outer, batched wk loads, 8 accumulators
            kps = [psA.tile([128, 512], F32, tag="pa", name=f"kps{i}")
                   for i in range(8)]
            for k in range(16):
                wt = wst.tile([128, 512], BF16, tag="wk")
                nc.sync.dma_start(wt[:], wkv[k * 128:(k + 1) * 128, 0:512])
                for fk in range(4):
                    nc.tensor.matmul(kps[fk * 2][:],
                                     wt[:, fk * 128:(fk + 1) * 128],
                                     xkv[k][:, 0:512],
                                     start=(k == 0), stop=(k == 15))
                    nc.tensor.matmul(kps[fk * 2 + 1][:],
                                     wt[:, fk * 128:(fk + 1) * 128],
                                     xkv[k][:, 512:1024],
                                     start=(k == 0), stop=(k == 15))
            for fk in range(4):
                ps0, ps1 = kps[fk * 2], kps[fk * 2 + 1]
                nc.vector.tensor_copy(KT[fk][0:64, 0:512], ps0[0:64, :])
                nc.scalar.copy(KT[fk][0:64, 1024:1536], ps0[64:128, :])
                nc.vector.tensor_copy(KT[fk][0:64, 512:1024], ps1[0:64, :])
                nc.scalar.copy(KT[fk][0:64, 1536:2048], ps1[64:128, :])

            # A2: V natural [t, f]; shared wv load per k, 8 accumulators
            vps = [[psA.tile([128, 512], F32, tag="pa", name=f"pvv{p}_{i}")
                    for i in range(4)] for p in range(P)]
            for k in range(16):
                wvt = wvp.tile([128, 512], BF16, tag="wv")
                nc.sync.dma_start(wvt[:], wkv[k * 128:(k + 1) * 128, 512:1024])
                for p in range(P):
                    for tt in range(4):
                        nc.tensor.matmul(
                            vps[p][tt][:],
                            xkv[k][:, p * 512 + tt * 128:p * 512 + (tt + 1) * 128],
                            wvt[:],
                            start=(k == 0), stop=(k == 15))
            for p in range(P):
                for tt in range(4):
                    ps = vps[p][tt]
                    va = VA[p][tt]
                    dst = va[:].rearrange("q (h c) -> q h c", c=65)[:, :, 0:64]
                    src = ps[:].rearrange("q (h c) -> q h c", c=64)
                    if (p * 4 + tt) % 2 == 0:
                        nc.vector.tensor_copy(dst, src)
                    else:
                        nc.scalar.copy(dst, src)

        # ---------- Phase A3: Q^T (uses xqT) ----------
        qtp = top.enter_context(tc.tile_pool(name="qtp", bufs=1))
        QT = [qtp.tile([64, 1024], BF16, tag=f"qt{j}", name=f"qt{j}")
              for j in range(16)]
        otp = top.enter_context(tc.tile_pool(name="otp", bufs=1))
        OT = [[otp.tile([128, 256], BF16, tag=f"ot{p}_{i}", name=f"ot{p}_{i}") for i in range(16)]
              for p in range(P)]

        with ExitStack() as ph:
            xqp = ph.enter_context(tc.tile_pool(name="xqp", bufs=1))
            wst = ph.enter_context(tc.tile_pool(name="wst2", bufs=6))
            psA = ph.enter_context(tc.tile_pool(name="psA2", bufs=8, space="PSUM"))

            xq = []
            for k in range(16):
                t = xqp.tile([128, 512], BF16, tag=f"xq{k}", name=f"xq{k}")
                nc.sync.dma_start(t[:], xqT[k * 128:(k + 1) * 128, :])
                xq.append(t)

            for fg in range(4):
                qps = [psA.tile([128, 512], F32, tag="pa", name=f"qps{fg}_{i}")
                       for i in range(4)]
                for k in range(16):
                    wt = wst.tile([128, 512], BF16, tag="wq")
                    nc.sync.dma_start(
                        wt[:], wq[k * 128:(k + 1) * 128, fg * 512:(fg + 1) * 512])
                    for fi in range(4):
                        nc.tensor.matmul(qps[fi][:],
                                         wt[:, fi * 128:(fi + 1) * 128],
                                         xq[k][:],
                                         start=(k == 0), stop=(k == 15))
                for fi in range(4):
                    fq = fg * 4 + fi
                    ps = qps[fi]
                    # pair-major: cols = [p0:(headA,headB) | p1:(headA,headB)]
                    nc.vector.tensor_copy(QT[fq][0:64, 0:256], ps[0:64, 0:256])
                    nc.scalar.copy(QT[fq][0:64, 256:512], ps[64:128, 0:256])
                    nc.vector.tensor_copy(QT[fq][0:64, 512:768], ps[0:64, 256:512])
                    nc.scalar.copy(QT[fq][0:64, 768:1024], ps[64:128, 256:512])

        # ---------- Phase B: attention ----------
        with ExitStack() as ph:
            ep = ph.enter_context(tc.tile_pool(name="ep", bufs=8))
            lp = ph.enter_context(tc.tile_pool(name="lp", bufs=4))
            qkps = ph.enter_context(tc.tile_pool(name="qkps", bufs=4, space="PSUM"))
            pvps = ph.enter_context(tc.tile_pool(name="pvps", bufs=2, space="PSUM"))
            rps = ph.enter_context(tc.tile_pool(name="rps", bufs=2, space="PSUM"))

            # deferred normalize state from the previous (j, p) iteration
            pending = []

            def finish(st):
                pv, r2, j, p = st
                rb = rps.tile([64, 512], F32, tag="rb")
                nc.tensor.matmul(rb[:, 0:256], on_sb[:], r2[0:1, 0:256],
                                 start=True, stop=True)
                nc.tensor.matmul(rb[:, 256:512], on_sb[:], r2[0:1, 256:512],
                                 start=True, stop=True)
                rsb = lp.tile([64, 512], F32, tag="rsb")
                nc.vector.tensor_copy(rsb[:], rb[:])
                hA, hB = 2 * j, 2 * j + 1
                for h, c0 in ((hA, 0), (hB, 256)):
                    ot = OT[p][h // 2]
                    ob = (h % 2) * 64
                    nc.vector.tensor_tensor(
                        ot[ob:ob + 64, :],
                        pv[0:64, c0:c0 + 256],
                        rsb[0:64, c0:c0 + 256],
                        mybir.AluOpType.mult)

            for j in range(16):
                n = j // 2          # kv head
                jk = n // 2         # KT tile
                nhalf = n % 2       # kv-head half within KT tile
                for p in range(P):
                    pv = pvps.tile([65, 512], F32, tag="pv")
                    es = []
                    for tt in range(4):
                        qk = qkps.tile([128, 512], F32, tag="qk")
                        cA = nhalf * 1024 + p * 512 + tt * 128
                        # both grouped heads (2j, 2j+1): pair-major QT slab
                        nc.tensor.matmul(
                            qk[:],
                            KT[jk][0:64, cA:cA + 128],
                            QT[j][0:64, p * 512:(p + 1) * 512],
                            start=True, stop=True)
                        e = ep.tile([128, 512], BF16, tag="e")
                        nc.scalar.activation(
                            e[:], qk[:], mybir.ActivationFunctionType.Exp,
                            scale=float(SCALE))
                        es.append(e)
                        if tt >= 1:
                            # PV lags QK by one t-tile so exp can overlap
                            nc.tensor.matmul(
                                pv[:, 0:512],
                                VA[p][tt - 1][:, n * 65:(n + 1) * 65],
                                es[tt - 1][:, 0:512],
                                start=(tt == 1), stop=False)
                    if pending:
                        finish(pending.pop())
                    nc.tensor.matmul(
                        pv[:, 0:512],
                        VA[p][3][:, n * 65:(n + 1) * 65],
                        es[3][:, 0:512],
                        start=False, stop=True)

                    # softmax denominators -> reciprocal (row layout)
                    l2 = lp.tile([1, 512], F32, tag="l2")
                    nc.vector.tensor_copy(l2[0:1, 0:256], pv[64:65, 0:256])
                    nc.vector.tensor_copy(l2[0:1, 256:512], pv[64:65, 256:512])
                    r2f = lp.tile([1, 512], F32, tag="r2f")
                    nc.vector.reciprocal_approx_fast(r2f[:], l2[:])
                    r2 = lp.tile([1, 512], F32R, tag="r2")
                    with nc.allow_low_precision(reason="softmax denom to f32r"):
                        nc.vector.tensor_copy(r2[:], r2f[:])
                    pending.append((pv, r2, j, p))
            finish(pending.pop())

        # ---------- Phase C: output projection ----------
        with ExitStack() as ph:
            wop = ph.enter_context(tc.tile_pool(name="wop", bufs=6))
            yst = ph.enter_context(tc.tile_pool(name="yst", bufs=4))
            psC = ph.enter_context(tc.tile_pool(name="psC", bufs=8, space="PSUM"))

            for nn in range(4):
                acc = [[psC.tile([128, 512], F32, tag="pc", name=f"pc{nn}_{m}") for m in range(2)]
                       for p in range(P)]
                for k in range(16):
                    wt = wop.tile([128, 512], BF16, tag="wo")
                    nc.sync.dma_start(
                        wt[:], wo[k * 128:(k + 1) * 128, nn * 512:(nn + 1) * 512])
                    for p in range(P):
                        for m in range(2):
                            nc.tensor.matmul(
                                acc[p][m][:],
                                OT[p][k][:, m * 128:(m + 1) * 128],
                                wt[:],
                                start=(k == 0), stop=(k == 15))
                for p in range(P):
                    for m in range(2):
                        yt = yst.tile([128, 512], F32, tag="yt")
                        if m == 0:
                            nc.vector.tensor_copy(yt[:], acc[p][m][:])
                        else:
                            nc.scalar.copy(yt[:], acc[p][m][:])
                        r0 = p * 256 + m * 128
                        nc.sync.dma_start(
                            y[r0:r0 + 128, nn * 512:(nn + 1) * 512], yt[:])

    nc.compile()
    return nc


def _get_nc():
    if "nc" not in _CACHE:
        _CACHE["nc"] = _build()
    return _CACHE["nc"]


def _shard_inputs(x, wq, wkv, wo):
    import ml_dtypes

    bf16 = ml_dtypes.bfloat16
    x = np.asarray(x, dtype=np.float32)
    wq_b = np.ascontiguousarray(wq, dtype=np.float32).astype(bf16)
    wkv_b = np.ascontiguousarray(wkv, dtype=np.float32).astype(bf16)
    wo_b = np.ascontiguousarray(wo, dtype=np.float32).astype(bf16)

    in_maps = []
    for c in range(N_CORES):
        xq_cols = []
        xkv_cols = []
        for p in range(P):
            pg = 2 * c + p
            b, v = pg // V, pg % V
            xq_cols.append(np.ascontiguousarray(x[b, v].T))
            xkv_cols.append(np.ascontiguousarray(
                np.concatenate([x[b, (v - 1) % V], x[b, (v + 1) % V]], axis=0).T))
        in_maps.append({
            "xqT": np.concatenate(xq_cols, axis=1).astype(bf16),
            "xkvT": np.concatenate(xkv_cols, axis=1).astype(bf16),
            "wq": wq_b, "wkv": wkv_b, "wo": wo_b,
            "ones1": np.ones((1, HD), np.float32),
        })
    return in_maps


def kernel(x, wq, wkv, wo):
    from concourse.bass_utils import run_bass_kernel_spmd

    nc = _get_nc()
    in_maps = _shard_inputs(x, wq, wkv, wo)
    res = run_bass_kernel_spmd(nc, in_maps, list(range(N_CORES)),
                               trace=False)
    out = np.empty((B, V, S, D), np.float32)
    for c in range(N_CORES):
        yc = res.results[c]["y"]
        for p in range(P):
            pg = 2 * c + p
            b, v = pg // V, pg % V
            out[b, v] = yc[p * S:(p + 1) * S]
    return out


# revision 8
# speedup vs baseline: 1.4602x; 1.1605x over previous
"""CrossViewAttention Trainium2 kernel (bf16).

Shards the B*V=16 (batch, view) attention instances across 8 NeuronCores
(2 per core, data-parallel; weights replicated). The circular neighbor
gather (views v-1, v+1) is resolved on the host when slicing per-core
inputs, so no device collectives are needed.

All matmul operands are bf16 (PSUM accumulation stays fp32), which
halves HBM traffic and weight-load time vs fp32r at the same 1
cycle/row multiply rate. Per core, for each of its 2 pairs:
  K^T = wk.T @ x_kv^T        V = x_kv @ wv  (natural layout, +ones col)
  Q^T = wq.T @ x^T
  scores^T[t,q] = K^T.T @ Q^T   per head (GQA: head h uses kv head h//4)
  E = exp(scale*scores^T)    (no max subtraction; scores are O(1))
  [O^T; l] = V_aug.T @ E     (ones column folds the softmax denominator)
  O^T *= broadcast(1/l)      (K=1 ones matmul broadcasts 1/l over hd)
  y = O @ wo
"""
import numpy as np

B, V, S, D = 2, 8, 256, 2048
NH, NKV, KVR = 32, 8, 2
HD = D // NH  # 64
G = NH // NKV  # 4
N_CORES = 8
P = 2  # pairs per core
SCALE = 1.0 / np.sqrt(HD)

_CACHE = {}


def _build():
    import concourse.bass as bass
    import concourse.tile as tile
    import concourse.mybir as mybir
    from concourse import bacc
    from contextlib import ExitStack

    F32 = mybir.dt.float32
    F32R = mybir.dt.float32r
    BF16 = mybir.dt.bfloat16

    nc = bacc.Bacc("TRN2", target_bir_lowering=False, debug=False,
                   num_devices=N_CORES)
    xqT = nc.dram_tensor("xqT", [D, P * S], BF16, kind="ExternalInput").ap()
    xkvT = nc.dram_tensor("xkvT", [D, P * 512], BF16, kind="ExternalInput").ap()
    wq = nc.dram_tensor("wq", [D, D], BF16, kind="ExternalInput").ap()
    wkv = nc.dram_tensor("wkv", [D, 1024], BF16, kind="ExternalInput").ap()
    wo = nc.dram_tensor("wo", [D, D], BF16, kind="ExternalInput").ap()
    y = nc.dram_tensor("y", [P * S, D], F32, kind="ExternalOutput").ap()

    with tile.TileContext(nc) as tc, ExitStack() as top:
        misc = top.enter_context(tc.tile_pool(name="misc", bufs=1))
        ktp = top.enter_context(tc.tile_pool(name="ktp", bufs=1))
        vp = top.enter_context(tc.tile_pool(name="vp", bufs=1))

        KT = [ktp.tile([64, 2048], BF16, tag=f"kt{i}", name=f"kt{i}") for i in range(4)]
        VA = [[vp.tile([128, 8 * 65], BF16, tag=f"va{p}_{t}", name=f"va{p}_{t}") for t in range(4)]
              for p in range(P)]
        # softmax-denominator ones columns of V_aug: set once
        for p in range(P):
            for t in range(4):
                od = VA[p][t][:].rearrange("q (h c) -> q h c", c=65)[:, :, 64:65]
                nc.gpsimd.memset(od, 1.0)

        # ---------- Phase A1/A2: K^T, V (uses xkvT; xkv resident) ----------
        with ExitStack() as ph:
            xkp = ph.enter_context(tc.tile_pool(name="xkp", bufs=1))
            wvp = ph.enter_context(tc.tile_pool(name="wvp", bufs=6))
            wst = ph.enter_context(tc.tile_pool(name="wst", bufs=6))
            psA = ph.enter_context(tc.tile_pool(name="psA", bufs=8, space="PSUM"))

            xkv = []
            for k in range(16):
                t = xkp.tile([128, 1024], BF16, tag=f"xkv{k}", name=f"xkv{k}")
                nc.sync.dma_start(t[:], xkvT[k * 128:(k + 1) * 128, :])
                xkv.append(t)

            # A1: K^T[f, t]; k outer, batched wk loads, 8 accumulators
            kps = [psA.tile([128, 512], F32, tag="pa", name=f"kps{i}")
                   for i in range(8)]
            for k in range(16):
                wt = wst.tile([128, 512], BF16, tag="wk")
                nc.sync.dma_start(wt[:], wkv[k * 128:(k + 1) * 128, 0:512])
                for fk in range(4):
                    nc.tensor.matmul(kps[fk * 2][:],
                                     wt[:, fk * 128:(fk + 1) * 128],
                                     xkv[k][:, 0:512],
                                     start=(k == 0), stop=(k == 15))
                    nc.tensor.matmul(kps[fk * 2 + 1][:],
                                     wt[:, fk * 128:(fk + 1) * 128],
                                     xkv[k][:, 512:1024],
                                     start=(k == 0), stop=(k == 15))
            for fk in range(4):
                ps0, ps1 = kps[fk * 2], kps[fk * 2 + 1]
                nc.vector.tensor_copy(KT[fk][0:64, 0:512], ps0[0:64, :])
                nc.scalar.copy(KT[fk][0:64, 1024:1536], ps0[64:128, :])
                nc.vector.tensor_copy(KT[fk][0:64, 512:1024], ps1[0:64, :])
                nc.scalar.copy(KT[fk][0:64, 1536:2048], ps1[64:128, :])

            # A2: V natural [t, f]; shared wv load per k, 8 accumulators
            vps = [[psA.tile([128, 512], F32, tag="pa", name=f"pvv{p}_{i}")
                    for i in range(4)] for p in range(P)]
            for k in range(16):
                wvt = wvp.tile([128, 512], BF16, tag="wv")
                nc.sync.dma_start(wvt[:], wkv[k * 128:(k + 1) * 128, 512:1024])
                for p in range(P):
                    for tt in range(4):
                        nc.tensor.matmul(
                            vps[p][tt][:],
                            xkv[k][:, p * 512 + tt * 128:p * 512 + (tt + 1) * 128],
                            wvt[:],
                            start=(k == 0), stop=(k == 15))
            for p in range(P):
                for tt in range(4):
                    ps = vps[p][tt]
                    va = VA[p][tt]
                    dst = va[:].rearrange("q (h c) -> q h c", c=65)[:, :, 0:64]
                    src = ps[:].rearrange("q (h c) -> q h c", c=64)
                    if (p * 4 + tt) % 2 == 0:
                        nc.vector.tensor_copy(dst, src)
                    else:
                        nc.scalar.copy(dst, src)

        # ---------- Phase A3 + B: Q^T interleaved with attention ----------
        # A3's fg-th weight slab produces QT[4fg..4fg+3]; the 8 attention
        # iterations that consume them are emitted right after, so tensor
        # matmuls of slab fg+1 fill the dependency stalls of attention fg.
        qtp = top.enter_context(tc.tile_pool(name="qtp", bufs=1))
        QT = [qtp.tile([64, 1024], BF16, tag=f"qt{j}", name=f"qt{j}")
              for j in range(16)]
        otp = top.enter_context(tc.tile_pool(name="otp", bufs=1))
        OT = [[otp.tile([128, 256], BF16, tag=f"ot{p}_{i}", name=f"ot{p}_{i}") for i in range(16)]
              for p in range(P)]

        with ExitStack() as ph:
            xqp = ph.enter_context(tc.tile_pool(name="xqp", bufs=1))
            wst = ph.enter_context(tc.tile_pool(name="wst2", bufs=6))
            psA = ph.enter_context(tc.tile_pool(name="psA2", bufs=4, space="PSUM"))
            ep = ph.enter_context(tc.tile_pool(name="ep", bufs=8))
            lp = ph.enter_context(tc.tile_pool(name="lp", bufs=4))
            qkps = ph.enter_context(tc.tile_pool(name="qkps", bufs=2, space="PSUM"))
            pvps = ph.enter_context(tc.tile_pool(name="pvps", bufs=2, space="PSUM"))

            xq = []
            for k in range(16):
                t = xqp.tile([128, 512], BF16, tag=f"xq{k}", name=f"xq{k}")
                nc.sync.dma_start(t[:], xqT[k * 128:(k + 1) * 128, :])
                xq.append(t)

            # deferred normalize state from the previous (j, p) iteration
            pending = []

            def finish(st):
                pv, r2f, j, p = st
                rsb = lp.tile([64, 512], F32, tag="rsb")
                nc.gpsimd.partition_broadcast(rsb[:], r2f[0:1, :])
                hA, hB = 2 * j, 2 * j + 1
                for h, c0 in ((hA, 0), (hB, 256)):
                    ot = OT[p][h // 2]
                    ob = (h % 2) * 64
                    nc.vector.tensor_tensor(
                        ot[ob:ob + 64, :],
                        pv[0:64, c0:c0 + 256],
                        rsb[0:64, c0:c0 + 256],
                        mybir.AluOpType.mult)

            def attn_iter(j, p):
                n = j // 2          # kv head
                jk = n // 2         # KT tile
                nhalf = n % 2       # kv-head half within KT tile
                pv = pvps.tile([65, 512], F32, tag="pv")
                es = []
                for tt in range(4):
                    qk = qkps.tile([128, 512], F32, tag="qk")
                    cA = nhalf * 1024 + p * 512 + tt * 128
                    # both grouped heads (2j, 2j+1): pair-major QT slab
                    nc.tensor.matmul(
                        qk[:],
                        KT[jk][0:64, cA:cA + 128],
                        QT[j][0:64, p * 512:(p + 1) * 512],
                        start=True, stop=True)
                    e = ep.tile([128, 512], BF16, tag="e")
                    nc.scalar.activation(
                        e[:], qk[:], mybir.ActivationFunctionType.Exp,
                        scale=float(SCALE))
                    es.append(e)
                    if tt >= 1:
                        # PV lags QK by one t-tile so exp can overlap
                        nc.tensor.matmul(
                            pv[:, 0:512],
                            VA[p][tt - 1][:, n * 65:(n + 1) * 65],
                            es[tt - 1][:, 0:512],
                            start=(tt == 1), stop=False)
                if pending:
                    finish(pending.pop())
                nc.tensor.matmul(
                    pv[:, 0:512],
                    VA[p][3][:, n * 65:(n + 1) * 65],
                    es[3][:, 0:512],
                    start=False, stop=True)

                # softmax denominators -> SBUF (custom-DVE recip can't read PSUM)
                l2 = lp.tile([1, 512], F32, tag="l2")
                nc.vector.tensor_copy(l2[:], pv[64:65, 0:512])
                r2f = lp.tile([1, 512], F32, tag="r2f")
                nc.vector.reciprocal_approx_fast(r2f[:], l2[:])
                pending.append((pv, r2f, j, p))

            for fg in range(4):
                qps = [psA.tile([128, 512], F32, tag="pa", name=f"qps{fg}_{i}")
                       for i in range(4)]
                for k in range(16):
                    wt = wst.tile([128, 512], BF16, tag="wq")
                    nc.sync.dma_start(
                        wt[:], wq[k * 128:(k + 1) * 128, fg * 512:(fg + 1) * 512])
                    for fi in range(4):
                        nc.tensor.matmul(qps[fi][:],
                                         wt[:, fi * 128:(fi + 1) * 128],
                                         xq[k][:],
                                         start=(k == 0), stop=(k == 15))
                for fi in range(4):
                    fq = fg * 4 + fi
                    ps = qps[fi]
                    # pair-major: cols = [p0:(headA,headB) | p1:(headA,headB)]
                    nc.vector.tensor_copy(QT[fq][0:64, 0:256], ps[0:64, 0:256])
                    nc.scalar.copy(QT[fq][0:64, 256:512], ps[64:128, 0:256])
                    nc.vector.tensor_copy(QT[fq][0:64, 512:768], ps[0:64, 256:512])
                    nc.scalar.copy(QT[fq][0:64, 768:1024], ps[64:128, 256:512])
                for j in range(4 * fg, 4 * fg + 4):
                    for p in range(P):
                        attn_iter(j, p)
            finish(pending.pop())

        # ---------- Phase C: output projection ----------
        with ExitStack() as ph:
            wop = ph.enter_context(tc.tile_pool(name="wop", bufs=6))
            yst = ph.enter_context(tc.tile_pool(name="yst", bufs=4))
            psC = ph.enter_context(tc.tile_pool(name="psC", bufs=8, space="PSUM"))

            for nn in range(4):
                acc = [[psC.tile([128, 512], F32, tag="pc", name=f"pc{nn}_{m}") for m in range(2)]
                       for p in range(P)]
                for k in range(16):
                    wt = wop.tile([128, 512], BF16, tag="wo")
                    nc.sync.dma_start(
                        wt[:], wo[k * 128:(k + 1) * 128, nn * 512:(nn + 1) * 512])
                    for p in range(P):
                        for m in range(2):
                            nc.tensor.matmul(
                                acc[p][m][:],
                                OT[p][k][:, m * 128:(m + 1) * 128],
                                wt[:],
                                start=(k == 0), stop=(k == 15))
                for p in range(P):
                    for m in range(2):
                        yt = yst.tile([128, 512], F32, tag="yt")
                        if m == 0:
                            nc.vector.tensor_copy(yt[:], acc[p][m][:])
                        else:
                            nc.scalar.copy(yt[:], acc[p][m][:])
                        r0 = p * 256 + m * 128
                        nc.sync.dma_start(
                            y[r0:r0 + 128, nn * 512:(nn + 1) * 512], yt[:])

    nc.compile()
    return nc


def _get_nc():
    if "nc" not in _CACHE:
        _CACHE["nc"] = _build()
    return _CACHE["nc"]


def _shard_inputs(x, wq, wkv, wo):
    import ml_dtypes

    bf16 = ml_dtypes.bfloat16
    x = np.asarray(x, dtype=np.float32)
    wq_b = np.ascontiguousarray(wq, dtype=np.float32).astype(bf16)
    wkv_b = np.ascontiguousarray(wkv, dtype=np.float32).astype(bf16)
    wo_b = np.ascontiguousarray(wo, dtype=np.float32).astype(bf16)

    in_maps = []
    for c in range(N_CORES):
        xq_cols = []
        xkv_cols = []
        for p in range(P):
            pg = 2 * c + p
            b, v = pg // V, pg % V
            xq_cols.append(np.ascontiguousarray(x[b, v].T))
            xkv_cols.append(np.ascontiguousarray(
                np.concatenate([x[b, (v - 1) % V], x[b, (v + 1) % V]], axis=0).T))
        in_maps.append({
            "xqT": np.concatenate(xq_cols, axis=1).astype(bf16),
            "xkvT": np.concatenate(xkv_cols, axis=1).astype(bf16),
            "wq": wq_b, "wkv": wkv_b, "wo": wo_b,
        })
    return in_maps


def kernel(x, wq, wkv, wo):
    from concourse.bass_utils import run_bass_kernel_spmd

    nc = _get_nc()
    in_maps = _shard_inputs(x, wq, wkv, wo)
    res = run_bass_kernel_spmd(nc, in_maps, list(range(N_CORES)),
                               trace=False)
    out = np.empty((B, V, S, D), np.float32)
    for c in range(N_CORES):
        yc = res.results[c]["y"]
        for p in range(P):
            pg = 2 * c + p
            b, v = pg // V, pg % V
            out[b, v] = yc[p * S:(p + 1) * S]
    return out


# revision 17
# speedup vs baseline: 1.7032x; 1.1664x over previous
"""CrossViewAttention Trainium2 kernel (bf16).

Shards the B*V=16 (batch, view) attention instances across 8 NeuronCores
(2 per core, data-parallel; weights replicated). The circular neighbor
gather (views v-1, v+1) is resolved on the host when slicing per-core
inputs, so no device collectives are needed.

All matmul operands are bf16 (PSUM accumulation stays fp32), which
halves HBM traffic and weight-load time vs fp32r at the same 1
cycle/row multiply rate. Per core, for each of its 2 pairs:
  K^T = wk.T @ x_kv^T        V = x_kv @ wv  (natural layout, +ones col)
  Q^T = wq.T @ x^T
  scores^T[t,q] = K^T.T @ Q^T   per head (GQA: head h uses kv head h//4)
  E = exp(scale*scores^T)    (no max subtraction; scores are O(1))
  [O^T; l] = V_aug.T @ E     (ones column folds the softmax denominator)
  O^T *= broadcast(1/l)      (K=1 ones matmul broadcasts 1/l over hd)
  y = O @ wo
"""
import numpy as np

B, V, S, D = 2, 8, 256, 2048
NH, NKV, KVR = 32, 8, 2
HD = D // NH  # 64
G = NH // NKV  # 4
N_CORES = 8
P = 2  # pairs per core
SCALE = 1.0 / np.sqrt(HD)

_CACHE = {}


def _build():
    import concourse.bass as bass
    import concourse.tile as tile
    import concourse.mybir as mybir
    from concourse import bacc
    from contextlib import ExitStack

    F32 = mybir.dt.float32
    F32R = mybir.dt.float32r
    BF16 = mybir.dt.bfloat16

    nc = bacc.Bacc("TRN2", target_bir_lowering=False, debug=False,
                   num_devices=N_CORES)
    xqT = nc.dram_tensor("xqT", [D, P * S], BF16, kind="ExternalInput").ap()
    xkvT = nc.dram_tensor("xkvT", [D, P * 512], BF16, kind="ExternalInput").ap()
    wq = nc.dram_tensor("wq", [D, D], BF16, kind="ExternalInput").ap()
    wkv = nc.dram_tensor("wkv", [D, 1024], BF16, kind="ExternalInput").ap()
    wo = nc.dram_tensor("wo", [D, D], BF16, kind="ExternalInput").ap()
    y = nc.dram_tensor("y", [P * S, D], BF16, kind="ExternalOutput").ap()

    with tile.TileContext(nc) as tc, ExitStack() as top:
        misc = top.enter_context(tc.tile_pool(name="misc", bufs=1))
        ktp = top.enter_context(tc.tile_pool(name="ktp", bufs=1))
        vp = top.enter_context(tc.tile_pool(name="vp", bufs=1))

        KT = [ktp.tile([64, 2048], BF16, tag=f"kt{i}", name=f"kt{i}") for i in range(4)]
        VA = [[vp.tile([128, 8 * 65], BF16, tag=f"va{p}_{t}", name=f"va{p}_{t}") for t in range(4)]
              for p in range(P)]
        # softmax-denominator ones columns of V_aug: set once
        for p in range(P):
            for t in range(4):
                od = VA[p][t][:].rearrange("q (h c) -> q h c", c=65)[:, :, 64:65]
                nc.gpsimd.memset(od, 1.0)

        # ---------- Phase A1/A2: K^T, V (uses xkvT; xkv resident) ----------
        with ExitStack() as ph:
            xkp = ph.enter_context(tc.tile_pool(name="xkp", bufs=1))
            wvp = ph.enter_context(tc.tile_pool(name="wvp", bufs=6))
            wst = ph.enter_context(tc.tile_pool(name="wst", bufs=6))
            psA = ph.enter_context(tc.tile_pool(name="psA", bufs=8, space="PSUM"))

            xkv = [xkp.tile([128, 1024], BF16, tag=f"xkv{k}", name=f"xkv{k}")
                   for k in range(16)]

            # A1: K^T[f, t]; k outer, batched wk loads, 8 accumulators.
            # xkv tile k's DMA is issued alongside wk tile k so the first
            # matmul only waits for two transfers, not the whole 4MB.
            kps = [psA.tile([128, 512], F32, tag="pa", name=f"kps{i}")
                   for i in range(8)]
            for k in range(16):
                nc.sync.dma_start(xkv[k][:], xkvT[k * 128:(k + 1) * 128, :])
                wt = wst.tile([128, 512], BF16, tag="wk")
                nc.sync.dma_start(wt[:], wkv[k * 128:(k + 1) * 128, 0:512])
                for fk in range(4):
                    nc.tensor.matmul(kps[fk * 2][:],
                                     wt[:, fk * 128:(fk + 1) * 128],
                                     xkv[k][:, 0:512],
                                     start=(k == 0), stop=(k == 15))
                    nc.tensor.matmul(kps[fk * 2 + 1][:],
                                     wt[:, fk * 128:(fk + 1) * 128],
                                     xkv[k][:, 512:1024],
                                     start=(k == 0), stop=(k == 15))
            wv_pre = []
            for kk in range(2):
                wvt = wvp.tile([128, 512], BF16, tag="wv", name=f"wvpre{kk}")
                nc.sync.dma_start(wvt[:], wkv[kk * 128:(kk + 1) * 128, 512:1024])
                wv_pre.append(wvt)
            for fk in range(4):
                ps0, ps1 = kps[fk * 2], kps[fk * 2 + 1]
                nc.vector.tensor_copy(KT[fk][0:64, 0:512], ps0[0:64, :])
                nc.scalar.copy(KT[fk][0:64, 1024:1536], ps0[64:128, :])
                nc.vector.tensor_copy(KT[fk][0:64, 512:1024], ps1[0:64, :])
                nc.scalar.copy(KT[fk][0:64, 1536:2048], ps1[64:128, :])

            # A2: V natural [t, f]; shared wv load per k, 8 accumulators
            vps = [[psA.tile([128, 512], F32, tag="pa", name=f"pvv{p}_{i}")
                    for i in range(4)] for p in range(P)]
            for k in range(16):
                if k < 2:
                    wvt = wv_pre[k]
                else:
                    wvt = wvp.tile([128, 512], BF16, tag="wv")
                    nc.sync.dma_start(
                        wvt[:], wkv[k * 128:(k + 1) * 128, 512:1024])
                for p in range(P):
                    for tt in range(4):
                        nc.tensor.matmul(
                            vps[p][tt][:],
                            xkv[k][:, p * 512 + tt * 128:p * 512 + (tt + 1) * 128],
                            wvt[:],
                            start=(k == 0), stop=(k == 15))
            for p in range(P):
                for tt in range(4):
                    ps = vps[p][tt]
                    va = VA[p][tt]
                    dst = va[:].rearrange("q (h c) -> q h c", c=65)[:, :, 0:64]
                    src = ps[:].rearrange("q (h c) -> q h c", c=64)
                    if (p * 4 + tt) % 2 == 0:
                        nc.vector.tensor_copy(dst, src)
                    else:
                        nc.scalar.copy(dst, src)

        # ---------- Phase A3 + B: Q^T interleaved with attention ----------
        # A3's fg-th weight slab produces QT[4fg..4fg+3]; the 8 attention
        # iterations that consume them are emitted right after, so tensor
        # matmuls of slab fg+1 fill the dependency stalls of attention fg.
        wop = top.enter_context(tc.tile_pool(name="wop", bufs=10))
        yst = top.enter_context(tc.tile_pool(name="yst", bufs=6))
        pre_wo = []
        qtp = top.enter_context(tc.tile_pool(name="qtp", bufs=1))
        QT = [qtp.tile([64, 1024], BF16, tag=f"qt{j}", name=f"qt{j}")
              for j in range(16)]
        otp = top.enter_context(tc.tile_pool(name="otp", bufs=1))
        OT = [[otp.tile([128, 256], BF16, tag=f"ot{p}_{i}", name=f"ot{p}_{i}") for i in range(16)]
              for p in range(P)]

        with ExitStack() as ph:
            xqp = ph.enter_context(tc.tile_pool(name="xqp", bufs=1))
            wst = ph.enter_context(tc.tile_pool(name="wst2", bufs=6))
            psA = ph.enter_context(tc.tile_pool(name="psA2", bufs=2, space="PSUM"))
            ep = ph.enter_context(tc.tile_pool(name="ep", bufs=8))
            lp = ph.enter_context(tc.tile_pool(name="lp", bufs=4))
            qkps = ph.enter_context(tc.tile_pool(name="qkps", bufs=4, space="PSUM"))
            pvps = ph.enter_context(tc.tile_pool(name="pvps", bufs=2, space="PSUM"))

            xq = []
            for k in range(16):
                t = xqp.tile([128, 512], BF16, tag=f"xq{k}", name=f"xq{k}")
                nc.sync.dma_start(t[:], xqT[k * 128:(k + 1) * 128, :])
                xq.append(t)

            # deferred normalize state from the previous (j, p) iteration
            pending = []

            def finish(st):
                pv, r2f, j, p = st
                rsb = lp.tile([64, 512], F32, tag="rsb")
                nc.gpsimd.partition_broadcast(rsb[:], r2f[0:1, :])
                hA, hB = 2 * j, 2 * j + 1
                for h, c0 in ((hA, 0), (hB, 256)):
                    ot = OT[p][h // 2]
                    ob = (h % 2) * 64
                    nc.vector.tensor_tensor(
                        ot[ob:ob + 64, :],
                        pv[0:64, c0:c0 + 256],
                        rsb[0:64, c0:c0 + 256],
                        mybir.AluOpType.mult)

            def a3_dma(sl, k):
                wt = wst.tile([128, 256], BF16, tag="wq")
                nc.sync.dma_start(
                    wt[:], wq[k * 128:(k + 1) * 128, sl * 256:(sl + 1) * 256])
                return wt

            def a3_mm(k, qps, wt):
                for fi in range(2):
                    nc.tensor.matmul(qps[fi][:],
                                     wt[:, fi * 128:(fi + 1) * 128],
                                     xq[k][:],
                                     start=(k == 0), stop=(k == 15))

            def a3_drain(sl, qps):
                for fi in range(2):
                    fq = sl * 2 + fi
                    ps = qps[fi]
                    # pair-major: cols = [p0:(headA,headB) | p1:(headA,headB)]
                    nc.vector.tensor_copy(QT[fq][0:64, 0:256], ps[0:64, 0:256])
                    nc.scalar.copy(QT[fq][0:64, 256:512], ps[64:128, 0:256])
                    nc.vector.tensor_copy(QT[fq][0:64, 512:768], ps[0:64, 256:512])
                    nc.scalar.copy(QT[fq][0:64, 768:1024], ps[64:128, 256:512])

            def attn_iter(j, p, filler):
                def fill():
                    s = next(filler, None)
                    if s is not None:
                        a3_mm(*s)

                n = j // 2          # kv head
                jk = n // 2         # KT tile
                nhalf = n % 2       # kv-head half within KT tile
                pv = pvps.tile([65, 512], F32, tag="pv")
                es = []
                for tt in range(4):
                    if tt >= 1:
                        fill()      # next Q-proj k-step plugs the exp window
                    qk = qkps.tile([128, 512], F32, tag="qk")
                    cA = nhalf * 1024 + p * 512 + tt * 128
                    # both grouped heads (2j, 2j+1): pair-major QT slab
                    nc.tensor.matmul(
                        qk[:],
                        KT[jk][0:64, cA:cA + 128],
                        QT[j][0:64, p * 512:(p + 1) * 512],
                        start=True, stop=True)
                    e = ep.tile([128, 512], BF16, tag="e")
                    nc.scalar.activation(
                        e[:], qk[:], mybir.ActivationFunctionType.Exp,
                        scale=float(SCALE))
                    es.append(e)
                    if tt >= 1:
                        # PV lags QK by one t-tile so exp can overlap
                        nc.tensor.matmul(
                            pv[:, 0:512],
                            VA[p][tt - 1][:, n * 65:(n + 1) * 65],
                            es[tt - 1][:, 0:512],
                            start=(tt == 1), stop=False)
                fill()
                if pending:
                    finish(pending.pop())
                nc.tensor.matmul(
                    pv[:, 0:512],
                    VA[p][3][:, n * 65:(n + 1) * 65],
                    es[3][:, 0:512],
                    start=False, stop=True)

                # softmax denominators -> SBUF (custom-DVE recip can't read PSUM)
                l2 = lp.tile([1, 512], F32, tag="l2")
                nc.vector.tensor_copy(l2[:], pv[64:65, 0:512])
                r2f = lp.tile([1, 512], F32, tag="r2f")
                nc.vector.reciprocal_approx_fast(r2f[:], l2[:])
                pending.append((pv, r2f, j, p))

            qps_cur = [psA.tile([128, 512], F32, tag="pa", name=f"qps0_{i}")
                       for i in range(2)]
            wts0 = [a3_dma(0, k) for k in range(3)]
            for k in range(16):
                if k + 3 < 16:
                    wts0.append(a3_dma(0, k + 3))
                a3_mm(k, qps_cur, wts0[k])
            a3_drain(0, qps_cur)
            for sl in range(8):
                if sl < 7:
                    qps_next = [psA.tile([128, 512], F32, tag="pa",
                                         name=f"qps{sl + 1}_{i}")
                                for i in range(2)]
                    wts = [a3_dma(sl + 1, k) for k in range(3)]

                    def gen(sl1, qps1, wts):
                        # wq DMA stays 3 k-steps ahead of its matmuls
                        for k in range(16):
                            if k + 3 < 16:
                                wts.append(a3_dma(sl1, k + 3))
                            yield (k, qps1, wts[k])

                    steps = gen(sl + 1, qps_next, wts)
                else:
                    steps = iter(())
                    # prefetch phase C's first weight slab during the B tail
                    for k in range(8):
                        wt = wop.tile([128, 512], BF16, tag="wo", name=f"wopre{k}")
                        nc.sync.dma_start(wt[:], wo[k * 128:(k + 1) * 128, 0:512])
                        pre_wo.append(wt)
                for j in range(2 * sl, 2 * sl + 2):
                    for p in range(P):
                        attn_iter(j, p, steps)
                for s in steps:
                    a3_mm(*s)
                if sl < 7:
                    a3_drain(sl + 1, qps_next)
            finish(pending.pop())

            # ---------- Phase C: output projection ----------
            # Runs inside the attention scope, accumulating in the qk/pa
            # PSUM rings so there is no pool-boundary barrier: each C
            # accumulator starts as soon as its bank's last attention
            # consumer (exp / drain copy) finishes.
            for nn in range(4):
                if nn % 2 == 0:
                    flat = [qkps.tile([128, 512], F32, tag="qk",
                                      name=f"pc{nn}_{i}") for i in range(4)]
                else:
                    flat = ([psA.tile([128, 512], F32, tag="pa",
                                      name=f"pc{nn}_{i}") for i in range(2)] +
                            [qkps.tile([128, 512], F32, tag="qk",
                                       name=f"pc{nn}_{i + 2}") for i in range(2)])
                acc = [[flat[0], flat[1]], [flat[2], flat[3]]]
                for k in range(16):
                    if nn == 0 and k < 8:
                        wt = pre_wo[k]
                    else:
                        wt = wop.tile([128, 512], BF16, tag="wo")
                        nc.sync.dma_start(
                            wt[:], wo[k * 128:(k + 1) * 128, nn * 512:(nn + 1) * 512])
                    for p in range(P):
                        for m in range(2):
                            nc.tensor.matmul(
                                acc[p][m][:],
                                OT[p][k][:, m * 128:(m + 1) * 128],
                                wt[:],
                                start=(k == 0), stop=(k == 15))
                for p in range(P):
                    for m in range(2):
                        yt = yst.tile([128, 512], BF16, tag="yt")
                        if m == 0:
                            nc.vector.tensor_copy(yt[:], acc[p][m][:])
                        else:
                            nc.scalar.copy(yt[:], acc[p][m][:])
                        r0 = p * 256 + m * 128
                        nc.sync.dma_start(
                            y[r0:r0 + 128, nn * 512:(nn + 1) * 512], yt[:])

    nc.compile()
    return nc
